# revision 4
# baseline (speedup 1.0000x reference)
"""Causal single-head attention on 8 Trainium2 NeuronCores.

Problem: B=4, S=2048, D_IN=1024, D_OUT=64 (fp32).
  Q = Xq @ Wq; K = Xk @ Wk; V = Xv @ Wv
  out = softmax(mask(Q K^T / 8)) @ V

Sharding: 8 cores = 4 batches x 2 interleaved query-block sets.
Core c handles batch b = c//2 and query blocks {128*(2t+h) : t in 0..7},
h = c%2.  The interleave balances causal work between the pair.

v2 design (vs the v1 baseline at ~40.4 us):
  * X is cast to bf16 AND pre-transposed on the host into the exact
    SBUF tile layout [128, 2, 8, 512] = [d%128, s-group, d-tile, s%512].
    Halves HBM traffic (6 MB/core vs 12 MB fp32) and removes all 192
    PE input transposes + their PSUM->SBUF copies.
  * One exchange collective instead of two: the K^T and V^T half
    projections live in one [64, 2, 1024] buffer and are
    pair-AllGathered in a single shot.
  * Tighter causal skipping: per k-tile only q-blocks >= kti//2 are
    computed; the boundary block is fixed up with a host-baked mask
    tile selected by kti parity (tri/zero for h=0, ones/tri for h=1).
    AV accumulates into 4x 256-wide PSUM chunks with sub-range matmuls
    at the boundary -- no stale-column memsets needed.
  * Attention for the first q-half is emitted before the second Q
    projection group so the PE stays busy during the tail DMA.

All loop structure is SPMD-uniform; h enters only via host-side data.
"""

import os
import numpy as np
import ml_dtypes

import concourse.bass as bass
import concourse.mybir as mybir
import concourse.tile as tile
from concourse.bass_utils import run_bass_kernel_spmd
from concourse.masks import make_identity
from concourse.vector_clock import ScopedClock

# ---------------------------------------------------------------------------
# Workaround: the walrus in this container rejects Tile's end-of-kernel drain
# when it carries >1 sem wait ("Too many sync wait commands").  Split the
# waits across single-wait SP NOPs placed just before the drain.
# ---------------------------------------------------------------------------


def _patched_drain_and_barrier(self, tick_clock, wait_clock):
    nc = self.nc
    collector = nc.sync.nop(nofuse=True)
    wait_clock.add_sem_waits(
        collector.ins, ScopedClock({None: tick_clock.global_clock})
    )
    si = collector.ins.sync_info
    waits = list(si.on_wait or []) if si is not None else []
    if si is not None:
        si.on_wait = waits[:1]
    for w in waits[1:]:
        n = nc.sync.nop(nofuse=True)
        nsi = n.ins.sync_info
        if nsi is None:
            n.ins.sync_info = mybir.SyncInfo(on_wait=[w], on_update=[])
        else:
            nsi.on_wait = [w]
    nc.sync.drain()
    nc.all_engine_barrier()
    assert self.sems is not None
    popped = nc._tile_sem_poison_stack.pop()
    assert popped is self._sem_poison
    nc.clear_and_free_semaphores(list(self.sems.allocated().values()))
    nc.all_engine_barrier()


tile.TileContext._drain_and_barrier = _patched_drain_and_barrier


def _split_sync_waits(nc, limit=1):
    """The nix walrus allows only `limit` sem waits per instruction; hoist
    extras onto same-engine NOPs placed immediately before the instruction."""
    ctr = [0]
    for fn in nc.m.functions:
        for bb in fn.blocks:
            out_list = []
            changed = False
            for inst in bb.instructions:
                si = inst.sync_info
                waits = list(si.on_wait) if si is not None and si.on_wait else []
                if len(waits) > limit:
                    keep = waits[-limit:]
                    for w in waits[:-limit]:
                        ctr[0] += 1
                        nop = mybir.InstNoOp(
                            name=f"waitsplit-{ctr[0]}",
                            engine=inst.engine,
                            ins=[],
                            outs=[],
                            sync_info=mybir.SyncInfo(on_wait=[w], on_update=[]),
                        )
                        out_list.append(nop)
                    si.on_wait = keep
                    changed = True
                out_list.append(inst)
            if changed:
                bb.instructions = out_list

# ---------------------------------------------------------------------------

B, S, D, E = 4, 2048, 1024, 64
SC = S // 2          # query rows per core
NT = SC // 128       # 8 local query blocks
NKT = S // 128       # 16 k-tiles
ND = D // 128        # 8 d-tiles
GROUP = 512          # s columns per projection group
NG = SC // GROUP     # 2 groups per tensor

F32 = mybir.dt.float32
BF16 = mybir.dt.bfloat16
EXP = mybir.ActivationFunctionType.Exp

# Unique-signature tag: the jax/neuron compile cache keys collide for
# same-signature modules, so every kernel variant carries a dummy input
# whose shape encodes the variant id.
KERNEL_UID = 121


def _build_nc(loop_reps=None, timing_mode=False, uid=KERNEL_UID):
    nc = bass.Bass()

    if timing_mode:
        # Internal (device-zeroed) X tensors: shrinks per-call transfer so
        # the K-rep wall-clock slope resolves the kernel's true exec time.
        xq = nc.dram_tensor("xq", (128, NG, ND, GROUP), BF16)
        xk = nc.dram_tensor("xk", (128, NG, ND, GROUP), BF16)
        xv = nc.dram_tensor("xv", (128, NG, ND, GROUP), BF16)
    else:
        xq = nc.dram_tensor("xq", (128, NG, ND, GROUP), BF16, kind="ExternalInput")
        xk = nc.dram_tensor("xk", (128, NG, ND, GROUP), BF16, kind="ExternalInput")
        xv = nc.dram_tensor("xv", (128, NG, ND, GROUP), BF16, kind="ExternalInput")
    wq = nc.dram_tensor("wq", (128, ND, E), BF16, kind="ExternalInput")
    wk = nc.dram_tensor("wk", (128, ND, E), BF16, kind="ExternalInput")
    wv = nc.dram_tensor("wv", (128, ND, E), BF16, kind="ExternalInput")
    # Boundary-tile masks, baked per-core on the host:  masks[:, j, :] is the
    # 0/1 tile applied to the q-block kti//2 at k-tile kti with j = kti % 2.
    #   h=0: j=0 -> lower-tri (diagonal), j=1 -> zeros (dead block)
    #   h=1: j=0 -> ones (fully live),    j=1 -> lower-tri (diagonal)
    masks = nc.dram_tensor("masks", (128, 2, 128), BF16, kind="ExternalInput")
    nc.dram_tensor("vtag", (1, uid), F32, kind="ExternalInput")
    out = nc.dram_tensor("out", (SC, E), F32, kind="ExternalOutput")

    with tile.TileContext(nc) as tc:
        with (
            tc.tile_pool(name="const", bufs=1) as cpool,
            tc.tile_pool(name="exp", bufs=3) as epool,
            tc.tile_pool(name="fin", bufs=2) as fpool,
            tc.tile_pool(name="ps_proj", bufs=1, space="PSUM") as ps_proj,
            tc.tile_pool(name="ps_sc", bufs=3, space="PSUM") as ps_sc,
            tc.tile_pool(name="ps_av", bufs=2, space="PSUM") as ps_av,
            tc.tile_pool(name="ps_tp", bufs=2, space="PSUM") as ps_tp,
            tc.tile_pool(name="dram", bufs=1, space="DRAM") as dpool,
        ):
            # ---- one-time constants ----
            ident_f = cpool.tile([128, 128], F32, tag="ident_f")
            make_identity(nc, ident_f)
            ident_b = cpool.tile([128, 128], BF16, tag="ident_b")
            make_identity(nc, ident_b)

            w_sb = {}
            for name, w in (("q", wq), ("k", wk), ("v", wv)):
                t = cpool.tile([128, ND, E], BF16, tag=f"w_{name}")
                nc.sync.dma_start(out=t[:], in_=w[:, :, :])
                w_sb[name] = t

            mask_sb = cpool.tile([128, 2, 128], BF16, tag="masks")
            nc.sync.dma_start(out=mask_sb[:], in_=masks[:, :, :])

            def emit_body():
                qt = cpool.tile([E, SC], BF16, tag="qt")
                kt = cpool.tile([E, S], BF16, tag="kt")
                vt = cpool.tile([E, S], BF16, tag="vt")
                # K^T half in [:, 0, :], V^T half in [:, 1, :]
                kvh = cpool.tile([E, 2, SC], BF16, tag="kvh")
                v1 = cpool.tile([128, NKT, E + 1], BF16, tag="v1")
                out_sb = cpool.tile([128, NT, E], F32, tag="out_sb")

                # ---- all X loads issued up-front on the SP (HWDGE) queue ----
                x_sb = {}
                for name, x_h in (("k", xk), ("v", xv), ("q", xq)):
                    t = cpool.tile([128, NG, ND, GROUP], BF16, tag=f"x_{name}")
                    for g in range(NG):
                        nc.sync.dma_start(out=t[:, g], in_=x_h[:, g])
                    x_sb[name] = t

                def project(name, dst, scale, g):
                    """One 512-column projection group: dst[:, g*512:(g+1)*512]
                    = (W^T X^T) slice in bf16 (optionally scaled)."""
                    xg = x_sb[name]
                    pps = ps_proj.tile([E, GROUP], F32, tag="proj")
                    for dt in range(ND):
                        nc.tensor.matmul(
                            pps[:],
                            w_sb[name][:, dt, :],
                            xg[:, g, dt, :],
                            start=(dt == 0),
                            stop=(dt == ND - 1),
                        )
                    sl = slice(g * GROUP, (g + 1) * GROUP)
                    if scale is None:
                        nc.scalar.copy(out=dst[:, sl], in_=pps[:])
                    else:
                        nc.scalar.mul(dst[:, sl], pps[:], scale)

                # ---- K/V half projections, then ONE pair exchange ----
                for g in range(NG):
                    project("k", kvh[:, 0, :], None, g)
                for g in range(NG):
                    project("v", kvh[:, 1, :], None, g)

                src_d = dpool.tile([E, 2, SC], BF16, tag="cc_src")
                dst_d = dpool.tile([2, E, 2, SC], BF16, tag="cc_dst")
                nc.gpsimd.dma_start(out=src_d[:], in_=kvh[:])
                nc.gpsimd.collective_compute(
                    "AllGather",
                    mybir.AluOpType.bypass,
                    replica_groups=[[0, 1], [2, 3], [4, 5], [6, 7]],
                    ins=[src_d[:]],
                    outs=[dst_d[:]],
                )
                nc.gpsimd.dma_start(
                    out=kt[:].rearrange("e (r s) -> e r s", r=2),
                    in_=dst_d[:, :, 0, :].rearrange("r e s -> e r s"),
                )
                nc.gpsimd.dma_start(
                    out=vt[:].rearrange("e (r s) -> e r s", r=2),
                    in_=dst_d[:, :, 1, :].rearrange("r e s -> e r s"),
                )

                # ---- first Q group, then V1 = [V | 1] via small transposes --
                project("q", qt, 1.0 / np.sqrt(E), 0)

                nc.vector.memset(v1[:], 1.0)
                for kti in range(NKT):
                    tps = ps_tp.tile([128, 128], BF16, tag="tp")
                    nc.tensor.transpose(
                        tps[:, 0:E],
                        vt[:, kti * 128 : (kti + 1) * 128],
                        ident_b[0:E, 0:E],
                    )
                    nc.vector.tensor_copy(out=v1[:, kti, 0:E], in_=tps[:, 0:E])

                def finalize(cp, av):
                    """Divide by the denominator row, transpose back, store."""
                    avsb = fpool.tile([E + 1, 256], F32, tag="avsb")
                    nc.vector.tensor_copy(out=avsb[:], in_=av[:])
                    for j in range(2):
                        t = 2 * cp + j
                        nps = ps_tp.tile([128, 128], F32, tag="tp")
                        nc.tensor.transpose(
                            nps[:, 0 : E + 1],
                            avsb[:, j * 128 : (j + 1) * 128],
                            ident_f[0 : E + 1, 0 : E + 1],
                        )
                        rec = fpool.tile([128, 1], F32, tag="rec")
                        nc.vector.reciprocal(rec[:], nps[:, E : E + 1])
                        nc.vector.tensor_scalar_mul(
                            out_sb[:, t, :], nps[:, 0:E], rec[:]
                        )
                    nc.sync.dma_start(
                        out=out[cp * 256 : (cp + 1) * 256, :].rearrange(
                            "(t p) e -> p t e", p=128
                        ),
                        in_=out_sb[:, 2 * cp : 2 * cp + 2, :],
                    )

                # ---- attention on q-column half c (cols c*512..c*512+511,
                # local q-blocks 4c..4c+3, AV chunks 2c and 2c+1) ----
                def attention_half(c):
                    avs = {
                        cp: ps_av.tile(
                            [E + 1, 256], F32, tag="av", name=f"av{cp}"
                        )
                        for cp in (2 * c, 2 * c + 1)
                    }
                    kti_hi = min(NKT, 8 * c + 8)  # k-tiles 0..kti_hi-1
                    for kti in range(kti_hi):
                        t0 = kti // 2  # h-agnostic first live q-block
                        w_off = min(max((t0 - 4 * c) * 128, 0), GROUP)
                        width = GROUP - w_off
                        assert width > 0
                        sps = ps_sc.tile([128, GROUP], F32, tag="sc")
                        nc.tensor.matmul(
                            sps[:, w_off:GROUP],
                            kt[:, kti * 128 : (kti + 1) * 128],
                            qt[:, c * GROUP + w_off : (c + 1) * GROUP],
                            start=True,
                            stop=True,
                        )
                        expt = epool.tile([128, GROUP], BF16, tag="expt")
                        nc.scalar.activation(
                            expt[:, w_off:GROUP], sps[:, w_off:GROUP], EXP
                        )
                        # boundary q-block fixup (host-baked per-core mask)
                        col = t0 * 128 - c * GROUP
                        if 0 <= col < GROUP:
                            nc.vector.tensor_mul(
                                expt[:, col : col + 128],
                                expt[:, col : col + 128],
                                mask_sb[:, kti % 2, :],
                            )
                        for cp in (2 * c, 2 * c + 1):
                            last = min(4 * cp + 3, kti_hi - 1)
                            if kti > last:
                                continue
                            sub = min(max((t0 - 2 * cp) * 128, 0), 256)
                            if sub >= 256:
                                continue
                            base = (cp % 2) * 256
                            nc.tensor.matmul(
                                avs[cp][:, sub:256],
                                v1[:, kti, :],
                                expt[:, base + sub : base + 256],
                                start=(kti == 0),
                                stop=(kti == last),
                                skip_group_check=True,
                            )
                            if kti == last:
                                finalize(cp, avs[cp])

                attention_half(0)
                project("q", qt, 1.0 / np.sqrt(E), 1)
                attention_half(1)

            if timing_mode:
                zt = cpool.tile([128, ND, GROUP], BF16, tag="zt")
                nc.vector.memset(zt[:], 0.0)
                for x_h in (xq, xk, xv):
                    for g in range(NG):
                        nc.sync.dma_start(out=x_h[:, g], in_=zt[:])

            for _rep in range(1 if loop_reps is None else loop_reps):
                emit_body()

    _split_sync_waits(nc)
    return nc


_CACHE = {}


def _get_nc():
    if "nc" not in _CACHE:
        _CACHE["nc"] = _build_nc()
    return _CACHE["nc"]


def _host_masks(h):
    """[128, 2, 128] boundary-tile masks for interleave h (see _build_nc)."""
    ki = np.arange(128)[:, None]
    qi = np.arange(128)[None, :]
    tri = (ki <= qi).astype(np.float32)
    m = np.empty((128, 2, 128), dtype=np.float32)
    if h == 0:
        m[:, 0, :] = tri
        m[:, 1, :] = 0.0
    else:
        m[:, 0, :] = 1.0
        m[:, 1, :] = tri
    return m.astype(ml_dtypes.bfloat16)


def _prep_xt(x_rows_f32):
    """[1024 s, 1024 d] fp32 -> bf16 [128, 2, 8, 512] = [d%128, g, dt, s%512]."""
    xb = x_rows_f32.astype(ml_dtypes.bfloat16)
    xb = xb.reshape(NG, GROUP, ND, 128)          # [g, s', dt, p]
    return np.ascontiguousarray(xb.transpose(3, 0, 2, 1))


def _prep_w(w_f32):
    """[1024, 64] fp32 -> bf16 [128, 8, 64] = [d%128, dt, e]."""
    wb = w_f32.astype(ml_dtypes.bfloat16)
    return np.ascontiguousarray(wb.reshape(ND, 128, E).transpose(1, 0, 2))


def kernel(**inputs):
    xq_full = np.asarray(inputs["inputs_for_queries"], dtype=np.float32)
    xk_full = np.asarray(inputs["inputs_for_keys"], dtype=np.float32)
    xv_full = np.asarray(inputs["inputs_for_values"], dtype=np.float32)
    wq = _prep_w(np.asarray(inputs["Weight_Q"], dtype=np.float32))
    wk = _prep_w(np.asarray(inputs["Weight_K"], dtype=np.float32))
    wv = _prep_w(np.asarray(inputs["Weight_V"], dtype=np.float32))

    nc = _get_nc()

    masks_h = [_host_masks(h) for h in (0, 1)]
    in_maps = []
    for c in range(8):
        b, h = c // 2, c % 2
        rows = np.concatenate(
            [np.arange((2 * t + h) * 128, (2 * t + h + 1) * 128) for t in range(NT)]
        )
        in_maps.append(
            {
                "xq": _prep_xt(xq_full[b][rows]),
                "xk": _prep_xt(xk_full[b][h * SC : (h + 1) * SC]),
                "xv": _prep_xt(xv_full[b][h * SC : (h + 1) * SC]),
                "wq": wq,
                "wk": wk,
                "wv": wv,
                "masks": masks_h[h],
                "vtag": np.zeros((1, KERNEL_UID), np.float32),
            }
        )

    trace = bool(int(os.environ.get("KERNEL_TRACE", "0")))
    res = run_bass_kernel_spmd(
        nc, in_maps, core_ids=list(range(8)), trace=trace
    )
    if trace:
        _CACHE["last_results"] = res

    out_full = np.empty((B, S, E), dtype=np.float32)
    for c in range(8):
        b, h = c // 2, c % 2
        oc = res.results[c]["out"]
        for t in range(NT):
            g = 2 * t + h
            out_full[b, g * 128 : (g + 1) * 128] = oc[t * 128 : (t + 1) * 128]
    return out_full


# revision 40
# speedup vs baseline: 4.0520x; 4.0520x over previous
"""Causal single-head attention on 8 Trainium2 NeuronCores.

Problem: B=4, S=2048, D_IN=1024, D_OUT=64 (fp32).
  Q = Xq @ Wq; K = Xk @ Wk; V = Xv @ Wv
  out = softmax(mask(Q K^T / 8)) @ V

Sharding: 8 cores = 4 batches x 2 interleaved query-block sets.
Core c handles batch b = c//2 and query blocks {128*(2t+h) : t in 0..7},
h = c%2.  The interleave balances causal work between the pair.

v5 design (vs the v1 baseline; ~277 -> ~265 engine instructions/call
vs ~800 in v1, ~4x faster measured end-to-end in this environment):
  * X is cast to bf16 AND pre-transposed on the host into the exact
    SBUF tile layout [128, g, 8, 512] = [d%128, s-group, d-tile, s%512].
    Halves HBM traffic (6 MB/core vs 12 MB fp32) and removes all 192
    PE input transposes + their PSUM->SBUF copies.
  * One exchange collective instead of two: the K^T and V^T half
    projections live in one [64, 2, 1024] buffer and are
    pair-AllGathered in a single shot.
  * V1 = [V | 1] is built with 16 hardware DMA-transposes straight from
    the gathered DRAM buffer (no PE transposes, no PSUM round trip).
  * Scores stay transposed (keys on partitions); exact causal skipping
    via t0 = kti//2 with a host-baked parity mask fixing up the
    boundary q-block (tri/zero for h=0, ones/tri for h=1), applied in
    pair-batched strided DVE ops.
  * AV^T accumulates in one [65, 512] PSUM bank per q-half with
    sub-range matmuls at the causal boundary; row 64 (from the ones
    column of V1) is the softmax denominator.
  * The [65, 512] AV^T blocks ship to the host as fp32; the host does
    the final (tiny, O(S*E)) transpose + divide in numpy.

All loop structure is SPMD-uniform; h enters only via host-side data.
"""

import os
import numpy as np
import ml_dtypes

import concourse.bass as bass
import concourse.mybir as mybir
import concourse.tile as tile
from concourse.bass_utils import run_bass_kernel_spmd
from concourse.masks import make_identity
from concourse.vector_clock import ScopedClock

# ---------------------------------------------------------------------------
# Workaround: the walrus in this container rejects Tile's end-of-kernel drain
# when it carries >1 sem wait ("Too many sync wait commands").  Split the
# waits across single-wait SP NOPs placed just before the drain.
# ---------------------------------------------------------------------------


def _patched_drain_and_barrier(self, tick_clock, wait_clock):
    nc = self.nc
    collector = nc.sync.nop(nofuse=True)
    wait_clock.add_sem_waits(
        collector.ins, ScopedClock({None: tick_clock.global_clock})
    )
    si = collector.ins.sync_info
    waits = list(si.on_wait or []) if si is not None else []
    if si is not None:
        si.on_wait = waits[:1]
    for w in waits[1:]:
        n = nc.sync.nop(nofuse=True)
        nsi = n.ins.sync_info
        if nsi is None:
            n.ins.sync_info = mybir.SyncInfo(on_wait=[w], on_update=[])
        else:
            nsi.on_wait = [w]
    nc.sync.drain()
    nc.all_engine_barrier()
    assert self.sems is not None
    popped = nc._tile_sem_poison_stack.pop()
    assert popped is self._sem_poison
    nc.clear_and_free_semaphores(list(self.sems.allocated().values()))
    nc.all_engine_barrier()


tile.TileContext._drain_and_barrier = _patched_drain_and_barrier


def _split_sync_waits(nc, limit=1):
    """The nix walrus allows only `limit` sem waits per instruction; hoist
    extras onto same-engine NOPs placed immediately before the instruction."""
    ctr = [0]
    for fn in nc.m.functions:
        for bb in fn.blocks:
            out_list = []
            changed = False
            for inst in bb.instructions:
                si = inst.sync_info
                waits = list(si.on_wait) if si is not None and si.on_wait else []
                if len(waits) > limit:
                    keep = waits[-limit:]
                    for w in waits[:-limit]:
                        ctr[0] += 1
                        nop = mybir.InstNoOp(
                            name=f"waitsplit-{ctr[0]}",
                            engine=inst.engine,
                            ins=[],
                            outs=[],
                            sync_info=mybir.SyncInfo(on_wait=[w], on_update=[]),
                        )
                        out_list.append(nop)
                    si.on_wait = keep
                    changed = True
                out_list.append(inst)
            if changed:
                bb.instructions = out_list

# ---------------------------------------------------------------------------

B, S, D, E = 4, 2048, 1024, 64
SC = S // 2          # query rows per core
NT = SC // 128       # 8 local query blocks
NKT = S // 128       # 16 k-tiles
ND = D // 128        # 8 d-tiles
GROUP = 512          # s columns per projection group
NG = SC // GROUP     # 2 groups per tensor

F32 = mybir.dt.float32
BF16 = mybir.dt.bfloat16
EXP = mybir.ActivationFunctionType.Exp

# Unique-signature tag: the jax/neuron compile cache keys collide for
# same-signature modules, so every kernel variant carries a dummy input
# whose shape encodes the variant id.
KERNEL_UID = 125


def _build_nc(
    loop_reps=None, timing_mode=False, use_cc=True, v1_dma=True, uid=KERNEL_UID
):
    nc = bass.Bass()

    # K/V source rows per core: half when the pair exchanges projections
    # via AllGather, full otherwise.
    ng_kv = NG if use_cc else 2 * NG

    # xkv packs the K then V source slices: slots 0:ng_kv = K, ng_kv: = V.
    # (Internal device-zeroed tensors in timing mode: shrinks the per-call
    # transfer so the K-rep wall-clock slope resolves true exec time.)
    if timing_mode:
        xq = nc.dram_tensor("xq", (128, NG, ND, GROUP), BF16)
        xkv = nc.dram_tensor("xkv", (128, 2 * ng_kv, ND, GROUP), BF16)
    else:
        xq = nc.dram_tensor("xq", (128, NG, ND, GROUP), BF16, kind="ExternalInput")
        xkv = nc.dram_tensor(
            "xkv", (128, 2 * ng_kv, ND, GROUP), BF16, kind="ExternalInput"
        )
    wq = nc.dram_tensor("wq", (128, ND, E), BF16, kind="ExternalInput")
    wk = nc.dram_tensor("wk", (128, ND, E), BF16, kind="ExternalInput")
    wv = nc.dram_tensor("wv", (128, ND, E), BF16, kind="ExternalInput")
    # Boundary-tile masks, baked per-core on the host; slot j = kti % 2:
    #   h=0: j=0 -> lower-tri (diagonal), j=1 -> zeros (dead block)
    #   h=1: j=0 -> ones (fully live),    j=1 -> lower-tri (diagonal)
    # duplicated over the 4 k-tile pairs of a q-half so the boundary fixup
    # of a whole half is ONE strided DVE multiply.
    masks = nc.dram_tensor("masks", (128, 8, 128), BF16, kind="ExternalInput")
    nc.dram_tensor("vtag", (1, uid), F32, kind="ExternalInput")
    # AV^T output: avh[c] = [V1^T exp^T](q-half c), rows 0:64 = V-weighted
    # sums, row 64 = softmax denominator.  Host divides + transposes.
    avh = nc.dram_tensor("avh", (2, E + 1, GROUP), F32, kind="ExternalOutput")

    with tile.TileContext(nc) as tc:
        with (
            tc.tile_pool(name="const", bufs=1) as cpool,
            tc.tile_pool(name="fin", bufs=2) as fpool,
            tc.tile_pool(name="ps_proj", bufs=1, space="PSUM") as ps_proj,
            tc.tile_pool(
                name="ps_sc", bufs=2 if (use_cc and v1_dma) else 1, space="PSUM"
            ) as ps_sc,
            tc.tile_pool(name="ps_av", bufs=1, space="PSUM") as ps_av,
            tc.tile_pool(name="ps_tp", bufs=2, space="PSUM") as ps_tp,
            tc.tile_pool(name="dram", bufs=1, space="DRAM") as dpool,
        ):
            # ---- one-time constants ----
            if not use_cc or not v1_dma:
                ident_b = cpool.tile([128, 128], BF16, tag="ident_b")
                make_identity(nc, ident_b)

            w_sb = {}
            for name, w in (("q", wq), ("k", wk), ("v", wv)):
                t = cpool.tile([128, ND, E], BF16, tag=f"w_{name}", name=f"w_{name}")
                nc.sync.dma_start(out=t[:], in_=w[:, :, :])
                w_sb[name] = t

            mask_sb = cpool.tile([128, 8, 128], BF16, tag="masks")
            nc.sync.dma_start(out=mask_sb[:], in_=masks[:, :, :])

            # V1 = [V | 1]: inner dim padded to 80 (160 B) so each k-tile
            # slot is 32-byte aligned for the xbar DMA transpose.  The ones
            # column (the softmax-denominator trick) is set once; the per-call
            # body only rewrites cols 0:64.
            v1 = cpool.tile([128, NKT, 80], BF16, tag="v1")
            nc.vector.memset(v1[:], 1.0)

            def emit_body():
                qt = cpool.tile([E, SC], BF16, tag="qt")
                kt = cpool.tile([E, S], BF16, tag="kt")
                if use_cc:
                    # K^T half in [:, 0, :], V^T half in [:, 1, :]
                    kvh = cpool.tile([E, 2, SC], BF16, tag="kvh")
                if not (use_cc and v1_dma):
                    vt = cpool.tile([E, S], BF16, tag="vt")
                # exp'd transposed scores, one slot per (q-half, k-tile):
                # slots 0:8 = half 0, 8:24 = half 1.  Columns are shifted so
                # slot position 0 is the causal-boundary q-block t0 = kti//2.
                exph = cpool.tile([128, 24, GROUP], BF16, tag="exph")

                # ---- all X loads issued up-front on the SP (HWDGE) queue ----
                xkv_sb = cpool.tile(
                    [128, 2 * ng_kv, ND, GROUP], BF16, tag="x_kv", name="x_kv"
                )
                nc.sync.dma_start(out=xkv_sb[:], in_=xkv[:])
                xq_sb = cpool.tile(
                    [128, NG, ND, GROUP], BF16, tag="x_q", name="x_q"
                )
                nc.sync.dma_start(out=xq_sb[:], in_=xq[:])
                x_sb = {"k": xkv_sb, "q": xq_sb}  # v = xkv slots ng_kv:

                def proj_mm(name, pt, slot, g):
                    """8 contract-tiled matmuls of one 512-column group into
                    PSUM slot `slot` (xkv slot `g`)."""
                    xg, w = x_sb.get(name, x_sb["k"]), w_sb[name]
                    if name == "v":
                        g = ng_kv + g
                    for dt in range(ND):
                        nc.tensor.matmul(
                            pt[:, slot, :],
                            w[:, dt, :],
                            xg[:, g, dt, :],
                            start=(dt == 0),
                            stop=(dt == ND - 1),
                        )

                def proj_copy(pt, dst, sl, scale):
                    # one plain-2D copy per 512-column group (a single merged
                    # 3D copy races its consumers -- see attention note)
                    ngr = (sl.stop - sl.start) // GROUP
                    for g in range(ngr):
                        dst_ap = dst[:, sl.start + g * GROUP : sl.start + (g + 1) * GROUP]
                        if scale is None:
                            nc.scalar.copy(out=dst_ap, in_=pt[:, g, :])
                        else:
                            nc.scalar.mul(dst_ap, pt[:, g, :], scale)

                # ---- K/V projections ----
                if use_cc:
                    # half projections, then ONE pair exchange
                    for name, col in (("k", 0), ("v", 1)):
                        pt = ps_proj.tile(
                            [E, 2, GROUP], F32, tag="proj", name=f"p_{name}"
                        )
                        for g in range(NG):
                            proj_mm(name, pt, g, g)
                        proj_copy(pt, kvh[:, col, :], slice(0, SC), None)

                    src_d = dpool.tile([E, 2, SC], BF16, tag="cc_src")
                    dst_d = dpool.tile([2, E, 2, SC], BF16, tag="cc_dst")
                    nc.gpsimd.dma_start(out=src_d[:], in_=kvh[:])
                    nc.gpsimd.collective_compute(
                        "AllGather",
                        mybir.AluOpType.bypass,
                        replica_groups=[[0, 1], [2, 3], [4, 5], [6, 7]],
                        ins=[src_d[:]],
                        outs=[dst_d[:]],
                    )
                    nc.gpsimd.dma_start(
                        out=kt[:].rearrange("e (r s) -> e r s", r=2),
                        in_=dst_d[:, :, 0, :].rearrange("r e s -> e r s"),
                    )
                    if not v1_dma:
                        nc.gpsimd.dma_start(
                            out=vt[:].rearrange("e (r s) -> e r s", r=2),
                            in_=dst_d[:, :, 1, :].rearrange("r e s -> e r s"),
                        )
                else:
                    # every core projects the full K and V itself
                    for name, dst in (("k", kt), ("v", vt)):
                        for gp in range(NG):
                            pt = ps_proj.tile(
                                [E, 2, GROUP], F32, tag="proj", name=f"p_{name}"
                            )
                            for g in range(2):
                                proj_mm(name, pt, g, 2 * gp + g)
                            proj_copy(
                                pt, dst, slice(2 * gp * GROUP, (2 * gp + 2) * GROUP),
                                None,
                            )

                # ---- Q projection (scale 1/sqrt(E) folded into wq on
                # the host) ----
                qpt = ps_proj.tile([E, 2, GROUP], F32, tag="proj", name="p_q")
                for g in range(NG):
                    proj_mm("q", qpt, g, g)
                proj_copy(qpt, qt, slice(0, SC), None)

                # ---- V1[:, kti, 0:64] = V k-tiles, natural layout ----
                if v1_dma:
                    # TWO multi-tile xbar DMA-transposes straight from the
                    # gathered DRAM buffer: [64, 1024] -> [128, 8, 64] writes
                    # v1[p, r*8+t, e] = V^T[e, r*1024 + t*128 + p].  Their 3D
                    # strided output escapes the tile dependency tracker, so
                    # two TRACKED single-tile transposes (slots 0 and 8)
                    # re-write the same data AFTER them on the in-order SP
                    # queue: the AV matmul for k-tile 0 (resp. 8) waits on
                    # the tracked write, and the in-order PE queue then
                    # covers every later v1 read.
                    for r in range(2):
                        nc.sync.dma_start(
                            out=v1[:, r * 8 : (r + 1) * 8, 0:E],
                            in_=dst_d[r, :, 1, :],
                            transpose=True,
                        )
                    for r in range(2):
                        nc.sync.dma_start(
                            out=v1[:, r * 8, 0:E],
                            in_=dst_d[r, :, 1, 0:128],
                            transpose=True,
                        )
                else:
                    nc.vector.memset(v1[:], 1.0)
                    for kti in range(NKT):
                        tps = ps_tp.tile([128, 128], BF16, tag="tp")
                        nc.tensor.transpose(
                            tps[:, 0:E],
                            vt[:, kti * 128 : (kti + 1) * 128],
                            ident_b[0:E, 0:E],
                        )
                        nc.vector.tensor_copy(out=v1[:, kti, 0:E], in_=tps[:, 0:E])

                # ---- attention on q-column half c (cols c*512..c*512+511,
                # local q-blocks 4c..4c+3) ----
                avs = [
                    ps_av.tile([E + 1, GROUP], F32, tag="av", name=f"av{i}", bufs=2)
                    for i in range(2)
                ]

                def attention_half(c):
                    slot0 = 8 * c  # exph slot base for this half
                    kti_hi = min(NKT, 8 * c + 8)  # k-tiles 0..kti_hi-1
                    for kti in range(kti_hi):
                        t0 = kti // 2  # h-agnostic first live q-block
                        w_off = min(max((t0 - 4 * c) * 128, 0), GROUP)
                        width = GROUP - w_off
                        # the (2a, 2a+1) k-tile pair shares t0 = a and one
                        # two-bank PSUM tile; exp runs pair-wide, writing the
                        # slots COLUMN-SHIFTED: position 0 = boundary block t0
                        if kti % 2 == 0:
                            sps = ps_sc.tile([128, 2, GROUP], F32, tag="sc")
                        nc.tensor.matmul(
                            sps[:, kti % 2, w_off:GROUP],
                            kt[:, kti * 128 : (kti + 1) * 128],
                            qt[:, c * GROUP + w_off : (c + 1) * GROUP],
                            start=True,
                            stop=True,
                        )
                        if kti % 2 == 1:
                            nc.scalar.activation(
                                exph[:, slot0 + kti - 1 : slot0 + kti + 1, 0:width],
                                sps[:, :, w_off:GROUP],
                                EXP,
                            )
                    # ONE boundary fixup for the whole half: the boundary
                    # q-block sits at position 0 of the 8 slots whose k-tiles
                    # carry it (half 0: ktis 0-7; half 1: ktis 8-15)
                    mslot = slot0 if c == 0 else slot0 + 8
                    nc.vector.tensor_mul(
                        exph[:, mslot : mslot + 8, 0:128],
                        exph[:, mslot : mslot + 8, 0:128],
                        mask_sb[:, :, :],
                    )
                    # AV^T accumulation; row 64 = softmax denominator
                    for kti in range(kti_hi):
                        t0 = kti // 2
                        w_off = min(max((t0 - 4 * c) * 128, 0), GROUP)
                        nc.tensor.matmul(
                            avs[c][:, w_off:GROUP],
                            v1[:, kti, 0 : E + 1],
                            exph[:, slot0 + kti, 0 : GROUP - w_off],
                            start=(kti == 0),
                            stop=(kti == kti_hi - 1),
                            skip_group_check=True,
                        )

                attention_half(0)
                attention_half(1)

                # ship AV^T (+ denominator row) to the host
                for c in range(2):
                    avsb = fpool.tile([E + 1, GROUP], F32, tag="avsb", name=f"avsb{c}")
                    nc.vector.tensor_copy(out=avsb[:], in_=avs[c][:])
                    nc.sync.dma_start(out=avh[c], in_=avsb[:])

            if timing_mode:
                zt = cpool.tile([128, ND, GROUP], BF16, tag="zt")
                nc.vector.memset(zt[:], 0.0)
                for x_h, ngt in ((xq, NG), (xkv, 2 * ng_kv)):
                    for g in range(ngt):
                        nc.sync.dma_start(out=x_h[:, g], in_=zt[:])

            for _rep in range(1 if loop_reps is None else loop_reps):
                emit_body()

    _split_sync_waits(nc)
    return nc


_CACHE = {}
USE_CC = True


def _get_nc():
    if "nc" not in _CACHE:
        _CACHE["nc"] = _build_nc(use_cc=USE_CC)
    return _CACHE["nc"]


def _host_masks(h):
    """[128, 8, 128] boundary-tile masks for interleave h (see _build_nc):
    the j = kti%2 pattern, duplicated over a q-half's 4 k-tile pairs."""
    ki = np.arange(128)[:, None]
    qi = np.arange(128)[None, :]
    tri = (ki <= qi).astype(np.float32)
    m = np.empty((128, 2, 128), dtype=np.float32)
    if h == 0:
        m[:, 0, :] = tri
        m[:, 1, :] = 0.0
    else:
        m[:, 0, :] = 1.0
        m[:, 1, :] = tri
    return np.ascontiguousarray(np.tile(m, (1, 4, 1))).astype(ml_dtypes.bfloat16)


def _prep_xt(x_rows_f32):
    """[n*512 s, 1024 d] fp32 -> bf16 [128, n, 8, 512] = [d%128, g, dt, s%512]."""
    ng = x_rows_f32.shape[0] // GROUP
    xb = x_rows_f32.astype(ml_dtypes.bfloat16)
    xb = xb.reshape(ng, GROUP, ND, 128)          # [g, s', dt, p]
    return np.ascontiguousarray(xb.transpose(3, 0, 2, 1))


def _prep_w(w_f32):
    """[1024, 64] fp32 -> bf16 [128, 8, 64] = [d%128, dt, e]."""
    wb = w_f32.astype(ml_dtypes.bfloat16)
    return np.ascontiguousarray(wb.reshape(ND, 128, E).transpose(1, 0, 2))


def kernel(**inputs):
    xq_full = np.asarray(inputs["inputs_for_queries"], dtype=np.float32)
    xk_full = np.asarray(inputs["inputs_for_keys"], dtype=np.float32)
    xv_full = np.asarray(inputs["inputs_for_values"], dtype=np.float32)
    # 1/sqrt(E) attention scale folded into the Q weights
    wq = _prep_w(np.asarray(inputs["Weight_Q"], dtype=np.float32) / np.sqrt(E))
    wk = _prep_w(np.asarray(inputs["Weight_K"], dtype=np.float32))
    wv = _prep_w(np.asarray(inputs["Weight_V"], dtype=np.float32))

    nc = _get_nc()

    masks_h = [_host_masks(h) for h in (0, 1)]
    in_maps = []
    for c in range(8):
        b, h = c // 2, c % 2
        rows = np.concatenate(
            [np.arange((2 * t + h) * 128, (2 * t + h + 1) * 128) for t in range(NT)]
        )
        if USE_CC:
            xk_c = xk_full[b][h * SC : (h + 1) * SC]
            xv_c = xv_full[b][h * SC : (h + 1) * SC]
        else:
            xk_c, xv_c = xk_full[b], xv_full[b]
        in_maps.append(
            {
                "xq": _prep_xt(xq_full[b][rows]),
                "xkv": np.ascontiguousarray(
                    np.concatenate([_prep_xt(xk_c), _prep_xt(xv_c)], axis=1)
                ),
                "wq": wq,
                "wk": wk,
                "wv": wv,
                "masks": masks_h[h],
                "vtag": np.zeros((1, KERNEL_UID), np.float32),
            }
        )

    trace = bool(int(os.environ.get("KERNEL_TRACE", "0")))
    res = run_bass_kernel_spmd(
        nc, in_maps, core_ids=list(range(8)), trace=trace
    )
    if trace:
        _CACHE["last_results"] = res

    # avh[c] = [65, 512] AV^T for q-half c: rows 0:64 are V-weighted sums,
    # row 64 the softmax denominator.  Final divide + transpose on host.
    out_full = np.empty((B, S, E), dtype=np.float32)
    for c in range(8):
        b, h = c // 2, c % 2
        av = res.results[c]["avh"]                        # [2, 65, 512]
        for half in range(2):
            num = av[half, 0:E, :]                        # [64, 512]
            den = av[half, E, :]                          # [512]
            blk = (num / den[None, :]).T                  # [512, 64] natural
            for j in range(4):
                t = 4 * half + j
                g = 2 * t + h
                out_full[b, g * 128 : (g + 1) * 128] = blk[
                    j * 128 : (j + 1) * 128
                ]
    return out_full
